# revision 19
# baseline (speedup 1.0000x reference)
"""Multi-head causal attention (B=4, S=2048, D=1024, H=16) on 8 TRN2 NeuronCores.

Sharding: data-parallel over batch (4) x tensor-parallel over heads (2 groups
of 8 heads). Core c handles batch c//2, head group c%2.

Per core, emitted interleaved per q-block j so the Tile scheduler overlaps
projection (PE-heavy) with attention (ACT-heavy):
  - K^T and V projections for s-block j (fp32r matmuls; V stored bf16 with a
    trailing ones column per head),
  - attention for q-block j: Q projection (fp32r), scores^T = K^T.T @ Q^T with
    the two heads of a pair row-packed into the 128-wide PE array, exp on ACT
    (no max-subtraction: scores ~N(0,1) here and softmax is shift-invariant),
    causal mask multiply on DVE (one op over both heads via a 0-step
    broadcast dim), bf16 PV accumulation where the ones column makes the
    softmax denominator fall out of the same matmuls, then normalize:
    DVE reciprocal -> GpSimd partition_broadcast -> DVE multiply (bf16 out),
  - output projection for s-block j (bf16, row-parallel partial sums).
Host adds the two partial [S, D] results per batch.
"""
import numpy as np
import ml_dtypes
import concourse.bass as bass
import concourse.mybir as mybir
import concourse.tile as tile
from concourse import bacc
from concourse.bass_utils import run_bass_kernel_spmd

B, S, D, H, DK = 4, 2048, 1024, 16, 64
EH = 512          # head columns per core (8 heads x 64)
QB = 512          # q-block size
KC = 128          # k-chunk size
HB = 256          # x half-block (s columns per streamed x tile)
NQ = S // QB
NKC = S // KC
NDC = D // 128
VW = DK + 1       # V columns per head incl. trailing ones column

f32 = mybir.dt.float32
f32r = mybir.dt.float32r
bf16 = mybir.dt.bfloat16

_cache = {}
LAST_NC = None
LAST_IN_MAPS = None


def _classify(mask2d):
    """Per (q-block, k-chunk): keep / skip / partial (dedup'd mask tiles).

    mask2d is [q, k] boolean; device mask tiles are [k, q] float (1=keep).
    """
    kept, partial, patterns, pat_idx = [], {}, [], {}
    for j in range(NQ):
        ks = []
        for c in range(NKC):
            sub = mask2d[j * QB:(j + 1) * QB, c * KC:(c + 1) * KC]
            if not sub.any():
                continue
            ks.append(c)
            if not sub.all():
                t = np.ascontiguousarray(sub.T).astype(np.float32)
                key = t.tobytes()
                if key not in pat_idx:
                    pat_idx[key] = len(patterns)
                    patterns.append(t)
                partial[(j, c)] = pat_idx[key]
        assert ks, f"q-block {j} fully masked; unsupported"
        kept.append(ks)
    if not patterns:
        patterns.append(np.ones((KC, QB), np.float32))
    assert len(patterns) <= 8, "too many distinct partial-mask patterns"
    qlos = []
    for p in patterns:
        cols = np.flatnonzero(p.any(axis=0))
        qlos.append(int(cols[0]) if len(cols) else 0)
    return kept, partial, np.stack(patterns), qlos


def _build(kept, partial, nu, qlos):
    nc = bacc.Bacc(None, target_bir_lowering=False)
    xT_d = nc.declare_dram_parameter("xT", [D, S], f32, isOutput=False)
    wq_d = nc.declare_dram_parameter("wq", [D, EH], f32, isOutput=False)
    wk_d = nc.declare_dram_parameter("wk", [D, EH], f32, isOutput=False)
    wv_d = nc.declare_dram_parameter("wv", [D, EH], f32, isOutput=False)
    wo_d = nc.declare_dram_parameter("wo", [EH, D], bf16, isOutput=False)
    masks_d = nc.declare_dram_parameter("masks", [nu, KC, QB], bf16, isOutput=False)
    ones8_d = nc.declare_dram_parameter("ones8", [128, 8], bf16, isOutput=False)
    y_d = nc.declare_dram_parameter("y", [S, D], f32, isOutput=True)

    Exp = mybir.ActivationFunctionType.Exp

    with tile.TileContext(nc) as tc, \
         nc.allow_low_precision(reason="fp32r/bf16 attention compute"), \
         tc.tile_pool(name="persist", bufs=1) as persist, \
         tc.tile_pool(name="wpers", bufs=1) as wpers, \
         tc.tile_pool(name="xp", bufs=2) as xp, \
         tc.tile_pool(name="ph2s", bufs=2) as ph2s, \
         tc.tile_pool(name="expp", bufs=4) as expp, \
         tc.tile_pool(name="ph3o", bufs=2) as ph3o, \
         tc.tile_pool(name="psA", bufs=1, space="PSUM") as psA:
        KT = [persist.tile([128, S], f32r, name=f"kt{t}") for t in range(4)]
        vaug = persist.tile([128, NKC, 8 * VW], bf16, name="vaug")
        aoT = [persist.tile([128, S], bf16, name=f"aot{t}") for t in range(4)]
        maskt = persist.tile([KC, nu, QB], bf16, name="maskt")
        wq = wpers.tile([128, NDC, EH], f32r, name="wq")
        wo = wpers.tile([128, 4, 2, QB], bf16, name="wo")
        wk = wpers.tile([128, NDC, EH], f32r, name="wk")
        wv = wpers.tile([128, NDC, EH], f32r, name="wv")

        # weight/mask loads: wk/wv (phase-1-critical) on the gpsimd queue,
        # the rest on the scalar queue; x streaming owns the sync queue.
        for dc_ in range(NDC):
            nc.gpsimd.dma_start(
                out=wk[:, dc_, :],
                in_=wk_d[dc_ * 128:(dc_ + 1) * 128, :].bitcast(f32r))
        for dc_ in range(NDC):
            nc.gpsimd.dma_start(
                out=wv[:, dc_, :],
                in_=wv_d[dc_ * 128:(dc_ + 1) * 128, :].bitcast(f32r))
        nc.scalar.dma_start(
            out=wq, in_=wq_d[:, :].rearrange("(dc p) e -> p dc e", p=128).bitcast(f32r))
        for u in range(nu):
            nc.scalar.dma_start(out=maskt[:, u, :], in_=masks_d[u, :, :])
        nc.scalar.dma_start(
            out=wo, in_=wo_d[:, :].rearrange("(t p) (db u) -> p t db u", p=128, u=QB))

        def proj_block(j, xt):
            """K^T and V projections for s-block j."""
            for t in range(4):
                pk = psA.tile([128, QB], f32, name="pk", tag="pk", bufs=1)
                for dc_ in range(NDC):
                    nc.tensor.matmul(
                        pk[:, :], wk[:, dc_, t * 128:(t + 1) * 128],
                        xt[:, dc_, :],
                        start=(dc_ == 0), stop=(dc_ == NDC - 1))
                nc.vector.tensor_copy(
                    KT[t][:, j * QB:(j + 1) * QB], pk[:, :].bitcast(f32r))
            for sv in range(4):
                ci = j * 4 + sv
                pvv = psA.tile([128, QB], f32, name="pvv", tag="pq", bufs=1)
                for dc_ in range(NDC):
                    nc.tensor.matmul(
                        pvv[:, :], xt[:, dc_, sv * 128:(sv + 1) * 128],
                        wv[:, dc_, :],
                        start=(dc_ == 0), stop=(dc_ == NDC - 1))
                nc.vector.tensor_copy(
                    vaug[:, ci, :].rearrange("p (h w) -> p h w", w=VW)[:, :, 0:DK],
                    pvv[:, :].rearrange("p (h w) -> p h w", w=DK))
                nc.gpsimd.dma_start(
                    out=vaug[:, ci, :].rearrange("p (h w) -> p h w", w=VW)[:, :, DK:VW],
                    in_=ones8_d[:, :].unsqueeze(-1))

        def qproj_block(j):
            """x streaming + Q projection for q-block j; returns (xts, qts)."""
            xt = xp.tile([128, NDC, QB], f32r, name="xt", tag="xt")
            for dc_ in range(NDC):
                nc.sync.dma_start(
                    out=xt[:, dc_, :],
                    in_=xT_d[dc_ * 128:(dc_ + 1) * 128, j * QB:(j + 1) * QB]
                        .bitcast(f32r))
            qts = []
            for t in range(4):
                pq = psA.tile([128, QB], f32, name="pq", tag="pq", bufs=1)
                for dc_ in range(NDC):
                    nc.tensor.matmul(
                        pq[:, :], wq[:, dc_, t * 128:(t + 1) * 128],
                        xt[:, dc_, :],
                        start=(dc_ == 0), stop=(dc_ == NDC - 1))
                qt = ph2s.tile([128, QB], f32r, name="qt", tag="qt", bufs=4)
                nc.vector.tensor_copy(qt[:, :], pq[:, :].bitcast(f32r))
                qts.append(qt)
            return xt, qts

        def attn_block(j, qts):
            ks = kept[j]
            C = len(ks)
            for t in range(4):
                qt = qts[t]
                pvA = psA.tile([VW, QB], f32, name="pvA", tag="pvA")
                pvB = psA.tile([VW, QB], f32, name="pvB", tag="pvB")
                for idx, c in enumerate(ks):
                    u = partial.get((j, c))
                    qlo = qlos[u] if u is not None else 0
                    # fp32r needs a moving dim >= 256 for full rate
                    slo = min(qlo, QB - 256)
                    sc = psA.tile([128, 2, QB], f32, name="sc", tag="sc", bufs=2)
                    nc.tensor.matmul(
                        sc[:, 0, slo:], KT[t][0:64, c * KC:(c + 1) * KC],
                        qt[0:64, slo:], start=True, stop=True)
                    nc.tensor.matmul(
                        sc[:, 1, slo:], KT[t][64:128, c * KC:(c + 1) * KC],
                        qt[64:128, slo:], start=True, stop=True)
                    et = expp.tile([128, 2, QB], bf16, name="et")
                    nc.scalar.activation(et[:, :, qlo:], sc[:, :, qlo:], Exp,
                                         scale=0.125)
                    if u is not None:
                        mk = maskt[:, u, qlo:]
                        mk2 = bass.AP(tensor=mk.tensor, offset=mk.offset,
                                      ap=[mk.ap[0], [0, 2], mk.ap[1]])
                        nc.vector.tensor_mul(et[:, :, qlo:], et[:, :, qlo:], mk2)
                    nc.tensor.matmul(
                        pvA[:, qlo:], vaug[:, c, VW * 2 * t:VW * (2 * t + 1)],
                        et[:, 0, qlo:], start=(idx == 0), stop=(idx == C - 1))
                    nc.tensor.matmul(
                        pvB[:, qlo:], vaug[:, c, VW * (2 * t + 1):VW * (2 * t + 2)],
                        et[:, 1, qlo:], start=(idx == 0), stop=(idx == C - 1))
                for pv, hb in ((pvA, 0), (pvB, 64)):
                    recip = ph2s.tile([1, QB], f32r, name="recip")
                    nc.vector.reciprocal(recip[:, :], pv[DK:VW, :])
                    bcs = ph2s.tile([DK, QB], f32r, name="bcs")
                    nc.gpsimd.partition_broadcast(bcs[:, :], recip[:, :])
                    nc.vector.tensor_mul(
                        aoT[t][hb:hb + DK, j * QB:(j + 1) * QB],
                        pv[0:DK, :].bitcast(f32r), bcs[:, :])

        def out_block(j):
            last = (j == NQ - 1)
            for sv in range(4):
                si = j * 4 + sv
                for db in range(2):
                    if last:
                        py = psA.tile([128, 2, QB], f32, name="py",
                                      tag="sc", bufs=2)[:, 0, :]
                    else:
                        py = psA.tile([128, QB], f32, name="py", tag="pq", bufs=1)
                    for t in range(4):
                        nc.tensor.matmul(
                            py[:, :], aoT[t][:, si * KC:(si + 1) * KC],
                            wo[:, t, db, :],
                            start=(t == 0), stop=(t == 3))
                    ys = ph3o.tile([128, QB], f32, name="ys")
                    nc.vector.tensor_copy(ys[:, :], py[:, :])
                    nc.gpsimd.dma_start(
                        out=y_d[si * KC:(si + 1) * KC, db * QB:(db + 1) * QB],
                        in_=ys[:, :])

        xt, qts = qproj_block(0)
        proj_block(0, xt)
        for j in range(NQ):
            attn_block(j, qts)
            if j + 1 < NQ:
                xt, qts = qproj_block(j + 1)
                proj_block(j + 1, xt)
            out_block(j)

    nc.finalize()
    return nc


def kernel(x, mask, w_qkv, w_out):
    global LAST_NC, LAST_IN_MAPS
    x = np.ascontiguousarray(np.asarray(x), dtype=np.float32)
    mask = np.asarray(mask)
    w_qkv = np.ascontiguousarray(np.asarray(w_qkv), dtype=np.float32)
    w_out = np.ascontiguousarray(np.asarray(w_out), dtype=np.float32)

    kept, partial, patterns, qlos = _classify(mask[0, 0])
    key = (tuple(tuple(k) for k in kept), tuple(sorted(partial.items())),
           len(patterns), tuple(qlos))
    nc = _cache.get(key)
    if nc is None:
        nc = _build(kept, partial, len(patterns), qlos)
        _cache[key] = nc

    ones8 = np.ones((128, 8), ml_dtypes.bfloat16)
    masks_bf16 = patterns.astype(ml_dtypes.bfloat16)
    in_maps = []
    for c in range(8):
        b, g = divmod(c, 2)
        in_maps.append({
            "xT": np.ascontiguousarray(x[b].T),
            "wq": np.ascontiguousarray(w_qkv[:, g * EH:(g + 1) * EH]),
            "wk": np.ascontiguousarray(w_qkv[:, D + g * EH:D + (g + 1) * EH]),
            "wv": np.ascontiguousarray(w_qkv[:, 2 * D + g * EH:2 * D + (g + 1) * EH]),
            "wo": np.ascontiguousarray(
                w_out[g * EH:(g + 1) * EH, :]).astype(ml_dtypes.bfloat16),
            "masks": masks_bf16,
            "ones8": ones8,
        })
    LAST_NC, LAST_IN_MAPS = nc, in_maps

    res = run_bass_kernel_spmd(nc, in_maps, core_ids=list(range(8)))
    y = np.empty((B, S, D), np.float32)
    for b in range(B):
        y[b] = res.results[2 * b]["y"] + res.results[2 * b + 1]["y"]
    return y
